# revision 38
# baseline (speedup 1.0000x reference)
"""Multi-head attention (B=2, S=2048, D=1024, H=16) on 8 trn2 NeuronCores.

Sharding: 2-way batch x 4-way head-group tensor parallel. Core c handles
batch c//4 and heads 4*(c%4) .. 4*(c%4)+3 (a 256-wide feature slice of the
q/k/v projections, and the matching row-slice of the out projection). Each
core emits a full-size [2048, 1024] bf16 partial of the output; the host
sums the 4 partials per batch and adds the output bias.

All matmul operands are bf16 (inputs/weights cast host-side; PSUM stays
f32). Softmax exp runs on ScalarE, optionally split with a DVE+GpSimd
Schraudolph bit-trick path (exp(x) ~ bitcast(i32(x*2^23/ln2 + magic))).

On-device dataflow (per core):
  - Q/K projected feature-major ([dq, t]); V token-major ([t, dv]) with 64
    ones columns appended so attn.V also yields the softmax denominator on
    psum partitions 64..127.
  - scoresT[k, q] per head via row-packed matmul pairs: head 2mh uses SBUF
    partitions 0..63, head 2mh+1 partitions 64..127; the two K=64 matmuls
    target disjoint PE row groups and run concurrently (2x throughput vs
    the zero-padded K=128 form).
  - exp on ScalarE (or Schraudolph on DVE+GpSimd) into PT bf16; attn.V
    accumulates over 16 k-tiles; rows 64..127 are the denominator;
    normalize via reciprocal_approx_fast + multiply on DVE.
  - The emission order software-pipelines everything: exp for q-chunk 0
    starts ~8us in, while the PE interleaves remaining projections,
    attn.V of earlier chunks, and the out-projection between score tiles
    so neither PE nor ScalarE ever starves.
"""

import ml_dtypes
import numpy as np

import concourse.bacc as bacc
import concourse.bass as bass
import concourse.mybir as mybir
import concourse.tile as tile
from concourse.bass_interp import get_hw_module
from concourse.bass_utils import run_bass_kernel_spmd

# problem constants (hardcoded; must match the reference)
B = 2
S = 2048
D = 1024
NH = 16
DH = 64
SCALE = DH ** -0.5

# sharding
N_CORES = 8
HG = 4                # heads per core
F = HG * DH           # 256 projected features per core
CH = 512              # token chunk
NCH = S // CH         # 4 chunks
P = 128
FT = D // P           # 8 feature tiles
MT = F // P           # 2 projected-feature tiles
KT = S // P           # 16 key-token tiles

f32 = mybir.dt.float32
i32 = mybir.dt.int32
i16 = mybir.dt.int16
bf16 = mybir.dt.bfloat16
EXP = mybir.ActivationFunctionType.Exp

# k-tiles whose exp runs on DVE via the Schraudolph bit trick (the rest
# run on ScalarE). The DVE writes i16(x*A + Bm) straight into an i16 PT
# tile; attn.V reads it bitcast to bf16 (same IEEE exponent layout, so
# the classic trick works at 2^7 scale) — no second pass needed.
SCH_KTS = (3, 7, 11, 15)
SCH_IDX = {kt: i for i, kt in enumerate(SCH_KTS)}
NSCH = len(SCH_KTS)
# exp(x) ~ bitcast_bf16(i16(x*A + Bm)); A folds in the softmax scale, Bm
# the Schraudolph magic with the half-ulp floor correction.
SCH_A = SCALE * (1 << 7) / np.log(2.0)
SCH_B = float((127 << 7) - 0.043677448 * (1 << 7) + 0.5)
ACT_KTS = [kt for kt in range(KT) if kt not in SCH_IDX]
ACT_IDX = {kt: i for i, kt in enumerate(ACT_KTS)}


def _emit(ctx, nc, tc, aps):
    xqT, xkT, xvT, wqT, wkT, wvT, woT, bq2, bk2, bv1, out = aps

    consts = ctx.enter_context(tc.tile_pool(name="consts", bufs=1))
    persist = ctx.enter_context(tc.tile_pool(name="persist", bufs=1))

    # weights / biases to SBUF. DMA issue order = transfer priority order:
    # the critical prefix (wk, biases, wq) on the scalar queue; wv/wo/bv
    # (needed only ~40us in) on the gpsimd queue behind xv.
    wk_sb = consts.tile([P, FT, F], bf16)
    wq_sb = consts.tile([P, FT, F], bf16)
    wv_sb = consts.tile([P, FT, F], bf16)
    wo_sb = consts.tile([P, MT, D], bf16)
    bq_sb = consts.tile([P, MT], f32)
    bk_sb = consts.tile([P, MT], f32)
    bv_sb = consts.tile([P, F], f32)
    nc.scalar.dma_start(out=wk_sb, in_=wkT)
    nc.scalar.dma_start(out=bq_sb, in_=bq2)
    nc.scalar.dma_start(out=bk_sb, in_=bk2)
    nc.scalar.dma_start(out=wq_sb, in_=wqT)

    # persistent activations
    QT_sb = persist.tile([P, MT, NCH, CH], bf16)   # [dq%128, dq//128, qc, q]
    KT_sb = persist.tile([P, MT, NCH, CH], bf16)   # same layout for K
    # V'' layout: [k%128, k//128, h, dv | 64 ones columns]
    V_sb = persist.tile([P, KT, HG, P], bf16)
    nc.vector.memset(V_sb[:, :, :, DH:P], 1.0)

    # input-chunk DMAs, in need order: xk0, xq0 first (sync queue); xv on
    # the gpsimd queue in parallel
    xk_t, xq_t, xv_t = [], [], []
    xT_pool = ctx.enter_context(tc.tile_pool(name="xT", bufs=2))

    def load_x(eng, x4_ap, c, tag, bufs=None):
        xT = xT_pool.tile([P, FT, CH], bf16, tag=tag, bufs=bufs)
        eng.dma_start(out=xT, in_=x4_ap[c])
        return xT

    xk_t.append(load_x(nc.sync, xkT, 0, "xk"))
    xq_t.append(load_x(nc.sync, xqT, 0, "xq", bufs=1))
    for c in range(1, NCH):
        xk_t.append(load_x(nc.sync, xkT, c, "xk"))
    for c in range(1, NCH):
        xq_t.append(load_x(nc.sync, xqT, c, "xq", bufs=1))
    xv_t.append(load_x(nc.gpsimd, xvT, 0, "xv"))
    nc.gpsimd.dma_start(out=bv_sb, in_=bv1.unsqueeze(0).to_broadcast((P, F)))
    nc.gpsimd.dma_start(out=wv_sb, in_=wvT)
    for c in range(1, NCH):
        xv_t.append(load_x(nc.gpsimd, xvT, c, "xv"))
    nc.gpsimd.dma_start(out=wo_sb, in_=woT)

    ps_proj = ctx.enter_context(
        tc.tile_pool(name="ps_proj", bufs=2, space="PSUM"))
    ps_s = ctx.enter_context(tc.tile_pool(name="ps_s", bufs=2, space="PSUM"))
    ps_o = ctx.enter_context(tc.tile_pool(name="ps_o", bufs=2, space="PSUM"))
    pt_pool = ctx.enter_context(tc.tile_pool(name="pt", bufs=3))
    pi_pool = ctx.enter_context(tc.tile_pool(name="pi", bufs=3))
    ot_pool = ctx.enter_context(tc.tile_pool(name="ot", bufs=2))
    ob_pool = ctx.enter_context(tc.tile_pool(name="ob", bufs=2))
    rc_pool = ctx.enter_context(tc.tile_pool(name="rc", bufs=1))

    pt_tiles = {}   # (qc, mh) -> bf16 PT tile [P, 2, len(ACT_KTS), CH]
    pi_tiles = {}   # (qc, mh) -> i16 Schraudolph PT tile [P, 2, NSCH, CH]
    ot_tiles = {}   # qc -> OT tile [P, MT, CH]

    def proj_qk(c, is_q, m):
        xT = (xq_t if is_q else xk_t)[c]
        tgt = QT_sb if is_q else KT_sb
        b_sb = bq_sb if is_q else bk_sb
        w_sb = wq_sb if is_q else wk_sb
        ps = ps_proj.tile([P, CH], f32, tag="proj")
        for ft in range(FT):
            nc.tensor.matmul(
                ps, w_sb[:, ft, m * P:(m + 1) * P], xT[:, ft, :],
                start=(ft == 0), stop=(ft == FT - 1),
            )
        nc.vector.tensor_scalar_add(tgt[:, m, c, :], ps, b_sb[:, m:m + 1])

    def proj_v(c, t4):
        xT = xv_t[c]
        ps = ps_proj.tile([P, F], f32, tag="proj")
        for ft in range(FT):
            nc.tensor.matmul(
                ps, xT[:, ft, t4 * P:(t4 + 1) * P], wv_sb[:, ft, :],
                start=(ft == 0), stop=(ft == FT - 1),
            )
        kt = c * (CH // P) + t4
        nc.vector.tensor_add(
            V_sb[:, kt, :, 0:DH],
            ps.rearrange("p (h d) -> p h d", h=HG),
            bv_sb.rearrange("p (h d) -> p h d", h=HG),
        )

    def scores(qc, mh, kts):
        for kt in kts:
            t0 = (kt % NCH) * P
            ps = ps_s.tile([P, 2, CH], f32, tag="s")
            nc.tensor.matmul(
                ps[:, 0, :], KT_sb[0:DH, mh, kt // NCH, t0:t0 + P],
                QT_sb[0:DH, mh, qc, :], start=True, stop=True,
            )
            nc.tensor.matmul(
                ps[:, 1, :], KT_sb[DH:P, mh, kt // NCH, t0:t0 + P],
                QT_sb[DH:P, mh, qc, :], start=True, stop=True,
            )
            if kt in SCH_IDX:
                nc.vector.tensor_scalar(
                    pi_tiles[(qc, mh)][:, :, SCH_IDX[kt], :], ps,
                    SCH_A, SCH_B,
                    op0=mybir.AluOpType.mult, op1=mybir.AluOpType.add,
                )
            else:
                nc.scalar.activation(
                    out=pt_tiles[(qc, mh)][:, :, ACT_IDX[kt], :], in_=ps,
                    func=EXP, scale=SCALE,
                )

    po_tiles = {}  # (qc, h) -> open psum accumulator for a split attn_v

    # accumulate the Schraudolph (i32/f32r) k-tiles first so their pi ring
    # slot frees one full unit before its next writer
    AV_ORDER = list(SCH_KTS) + ACT_KTS

    def attn_v_part(qc, h, i0, i1):
        pt = pt_tiles[(qc, h // 2)]
        pi = pi_tiles[(qc, h // 2)]
        if i0 == 0:
            po_tiles[(qc, h)] = ps_o.tile([P, CH], f32, tag="o",
                                          name=f"po{qc}_{h}")
        po = po_tiles[(qc, h)]
        for i in range(i0, i1):
            kt = AV_ORDER[i]
            if kt in SCH_IDX:
                mv = pi[:, h % 2, SCH_IDX[kt], :].bitcast(bf16)
            else:
                mv = pt[:, h % 2, ACT_IDX[kt], :]
            nc.tensor.matmul(
                po, V_sb[:, kt, h, :], mv,
                start=(i == 0), stop=(i == KT - 1),
                skip_group_check=True,
            )
        if i1 < KT:
            return
        mh, p0 = divmod(h, 2)
        p0 *= DH
        rs = rc_pool.tile([DH, CH], f32, tag="rs")
        rc = rc_pool.tile([DH, CH], f32, tag="rc")
        nc.vector.tensor_copy(rs, po[DH:P, :])
        nc.vector.reciprocal_approx_fast(rc, rs)
        nc.vector.tensor_mul(ot_tiles[qc][p0:p0 + DH, mh, :], po[0:DH, :], rc)

    def attn_v(qc, h):
        attn_v_part(qc, h, 0, KT)

    def out_proj(qc, t4):
        ot = ot_tiles[qc]
        ob = ob_pool.tile([P, D], bf16, tag="ob")
        for n2 in range(D // CH):
            ps = ps_proj.tile([P, CH], f32, tag="proj")
            for m in range(MT):
                nc.tensor.matmul(
                    ps, ot[:, m, t4 * P:(t4 + 1) * P],
                    wo_sb[:, m, n2 * CH:(n2 + 1) * CH],
                    start=(m == 0), stop=(m == MT - 1),
                )
            nc.vector.tensor_copy(ob[:, n2 * CH:(n2 + 1) * CH], ps)
        tt = qc * NCH + t4
        nc.sync.dma_start(out=out[tt * P:(tt + 1) * P, :], in_=ob)

    def new_pt(qc, mh):
        pt_tiles[(qc, mh)] = pt_pool.tile([P, 2, len(ACT_KTS), CH], bf16,
                                          tag="pt", name=f"pt{qc}_{mh}")
        pi_tiles[(qc, mh)] = pi_pool.tile([P, 2, NSCH, CH], i16,
                                          tag="pi", name=f"pi{qc}_{mh}")

    def new_ot(qc):
        ot_tiles[qc] = ot_pool.tile([P, MT, CH], bf16, tag="ot",
                                    name=f"ot{qc}")

    # ---- software-pipelined emission order ----
    # Pipeline unit = (qc, mh): 16 score-tile pairs feeding 11 exp
    # ACTIVATEs (ScalarE) + 5 Schraudolph tensor_scalars (DVE). Score
    # pieces of 2 k-tiles alternate with ~0.9us filler pieces (projection
    # halves, attn_v halves, out-proj halves) so the in-order PE queue
    # never runs far ahead of the psum-ring consumers.
    kh = lambda c, m: (lambda: proj_qk(c, False, m))
    qh = lambda c, m: (lambda: proj_qk(c, True, m))
    vh = lambda c, t4: (lambda: proj_v(c, t4))
    sc2 = lambda qc, mh, j: (lambda: scores(qc, mh, range(2 * j, 2 * j + 2)))
    avp = lambda qc, h, i0, i1: (lambda: attn_v_part(qc, h, i0, i1))
    op = lambda qc, t4: (lambda: out_proj(qc, t4))
    npt = lambda qc, mh: (lambda: new_pt(qc, mh))
    not_ = lambda qc: (lambda: new_ot(qc))

    schedule = [
        kh(0, 0), qh(0, 0), npt(0, 0), not_(0),
        # unit (0,0): remaining K halves + Q0 m1 as filler. Chunk c's m1
        # half precedes chunk c+2's m0 so the 2-deep xk ring never cycles.
        sc2(0, 0, 0), kh(1, 0), sc2(0, 0, 1), kh(0, 1),
        sc2(0, 0, 2), kh(2, 0), sc2(0, 0, 3), kh(1, 1),
        sc2(0, 0, 4), kh(3, 0), sc2(0, 0, 5), kh(2, 1),
        sc2(0, 0, 6), kh(3, 1), sc2(0, 0, 7), qh(0, 1),
        npt(0, 1),
        # unit (0,1): all of V projected here so attn_v can start in (1,0)
        sc2(0, 1, 0), vh(0, 0), vh(0, 1), sc2(0, 1, 1), vh(0, 2), vh(0, 3),
        sc2(0, 1, 2), vh(1, 0), vh(1, 1), sc2(0, 1, 3), vh(1, 2), vh(1, 3),
        sc2(0, 1, 4), vh(2, 0), vh(2, 1), sc2(0, 1, 5), vh(2, 2), vh(2, 3),
        sc2(0, 1, 6), vh(3, 0), vh(3, 1), sc2(0, 1, 7), vh(3, 2), vh(3, 3),
        qh(1, 0), qh(1, 1), npt(1, 0), not_(1),
        # unit (1,0)
        sc2(1, 0, 0), avp(0, 0, 0, 8), sc2(1, 0, 1), avp(0, 0, 8, 16),
        sc2(1, 0, 2), avp(0, 1, 0, 8), sc2(1, 0, 3), avp(0, 1, 8, 16),
        sc2(1, 0, 4), qh(2, 0), sc2(1, 0, 5), qh(2, 1),
        sc2(1, 0, 6), sc2(1, 0, 7), npt(1, 1),
        # unit (1,1)
        sc2(1, 1, 0), avp(0, 2, 0, 8), sc2(1, 1, 1), avp(0, 2, 8, 16),
        sc2(1, 1, 2), avp(0, 3, 0, 8), sc2(1, 1, 3), avp(0, 3, 8, 16),
        sc2(1, 1, 4), op(0, 0), op(0, 1), sc2(1, 1, 5), op(0, 2), op(0, 3),
        sc2(1, 1, 6), sc2(1, 1, 7), npt(2, 0), not_(2),
        # unit (2,0)
        sc2(2, 0, 0), avp(1, 0, 0, 8), sc2(2, 0, 1), avp(1, 0, 8, 16),
        sc2(2, 0, 2), avp(1, 1, 0, 8), sc2(2, 0, 3), avp(1, 1, 8, 16),
        sc2(2, 0, 4), qh(3, 0), sc2(2, 0, 5), qh(3, 1),
        sc2(2, 0, 6), sc2(2, 0, 7), npt(2, 1),
        # unit (2,1)
        sc2(2, 1, 0), avp(1, 2, 0, 8), sc2(2, 1, 1), avp(1, 2, 8, 16),
        sc2(2, 1, 2), avp(1, 3, 0, 8), sc2(2, 1, 3), avp(1, 3, 8, 16),
        sc2(2, 1, 4), op(1, 0), op(1, 1), sc2(2, 1, 5), op(1, 2), op(1, 3),
        sc2(2, 1, 6), sc2(2, 1, 7), npt(3, 0), not_(3),
        # unit (3,0)
        sc2(3, 0, 0), avp(2, 0, 0, 8), sc2(3, 0, 1), avp(2, 0, 8, 16),
        sc2(3, 0, 2), avp(2, 1, 0, 8), sc2(3, 0, 3), avp(2, 1, 8, 16),
        sc2(3, 0, 4), sc2(3, 0, 5), sc2(3, 0, 6), sc2(3, 0, 7), npt(3, 1),
        # unit (3,1): attn_v(3, 0/1) overlap exp(3, mh1)
        sc2(3, 1, 0), avp(2, 2, 0, 8), sc2(3, 1, 1), avp(2, 2, 8, 16),
        sc2(3, 1, 2), avp(2, 3, 0, 8), sc2(3, 1, 3), avp(2, 3, 8, 16),
        sc2(3, 1, 4), avp(3, 0, 0, 8), sc2(3, 1, 5), avp(3, 0, 8, 16),
        sc2(3, 1, 6), avp(3, 1, 0, 8), sc2(3, 1, 7), avp(3, 1, 8, 16),
        op(2, 0), op(2, 1), op(2, 2), op(2, 3),
        # tail: attn_v(3, 2/3) accumulate Schraudolph + early-ACT k-tiles
        # first so only the last exp tiles trail
        avp(3, 2, 0, 12), avp(3, 3, 0, 12),
        avp(3, 2, 12, KT), avp(3, 3, 12, KT),
        op(3, 0), op(3, 1), op(3, 2), op(3, 3),
    ]
    for unit in schedule:
        unit()


def _build():
    nc = bacc.Bacc("TRN2", target_bir_lowering=False, debug=False)
    # x inputs chunk-major [c, p, ft, t]; weights partition-major — every
    # DMA reads fully contiguous per-partition lines (host prepares these)
    xqT = nc.dram_tensor("xqT", [NCH, P, FT, CH], bf16,
                         kind="ExternalInput").ap()
    xkT = nc.dram_tensor("xkT", [NCH, P, FT, CH], bf16,
                         kind="ExternalInput").ap()
    xvT = nc.dram_tensor("xvT", [NCH, P, FT, CH], bf16,
                         kind="ExternalInput").ap()
    wqT = nc.dram_tensor("wqT", [P, FT, F], bf16, kind="ExternalInput").ap()
    wkT = nc.dram_tensor("wkT", [P, FT, F], bf16, kind="ExternalInput").ap()
    wvT = nc.dram_tensor("wvT", [P, FT, F], bf16, kind="ExternalInput").ap()
    woT = nc.dram_tensor("woT", [P, MT, D], bf16, kind="ExternalInput").ap()
    bq2 = nc.dram_tensor("bq2", [P, MT], f32, kind="ExternalInput").ap()
    bk2 = nc.dram_tensor("bk2", [P, MT], f32, kind="ExternalInput").ap()
    bv1 = nc.dram_tensor("bv1", [F], f32, kind="ExternalInput").ap()
    out = nc.dram_tensor("out", [S, D], bf16, kind="ExternalOutput").ap()
    from contextlib import ExitStack

    with tile.TileContext(nc) as tc, ExitStack() as ctx:
        _emit(ctx, nc, tc,
              (xqT, xkT, xvT, wqT, wkT, wvT, woT, bq2, bk2, bv1, out))
    nc.compile()
    nc.m = get_hw_module(nc.m)
    return nc


_cached_nc = None


def _get_nc():
    global _cached_nc
    if _cached_nc is None:
        _cached_nc = _build()
    return _cached_nc


def make_in_maps(query, key, value, Wq, bq, Wk, bk, Wv, bv, Wo, bo):
    query, key, value, Wq, bq, Wk, bk, Wv, bv, Wo = (
        np.asarray(a, np.float32)
        for a in (query, key, value, Wq, bq, Wk, bk, Wv, bv, Wo)
    )
    bff = ml_dtypes.bfloat16

    def x4(a, b):
        # [S, D] -> chunk-major [NCH, P, FT, CH] of a[b].T
        aT = a[b].T.reshape(FT, P, NCH, CH)
        return np.ascontiguousarray(aT.transpose(2, 1, 0, 3)).astype(bff)

    def w3(W, fs):
        # W[fs] is [F, D]; -> [P, FT, F] of W[fs].T
        wT = W[fs].T.reshape(FT, P, F)
        return np.ascontiguousarray(wT.transpose(1, 0, 2)).astype(bff)

    xTs = [
        tuple(x4(a, b) for a in (query, key, value)) for b in range(B)
    ]
    in_maps = []
    for c in range(N_CORES):
        b, g = divmod(c, 4)
        fs = slice(g * F, (g + 1) * F)
        qT, kT, vT = xTs[b]
        woT = Wo[:, fs].T.reshape(MT, P, D)
        in_maps.append({
            "xqT": qT,
            "xkT": kT,
            "xvT": vT,
            "wqT": w3(Wq, fs),
            "wkT": w3(Wk, fs),
            "wvT": w3(Wv, fs),
            "woT": np.ascontiguousarray(woT.transpose(1, 0, 2)).astype(bff),
            "bq2": np.ascontiguousarray(bq[fs].reshape(MT, P).T),
            "bk2": np.ascontiguousarray(bk[fs].reshape(MT, P).T),
            "bv1": np.ascontiguousarray(bv[fs]),
        })
    return in_maps


def combine_outputs(core_outs, bo):
    bo = np.asarray(bo, np.float32)
    out = np.empty((B, S, D), np.float32)
    for b in range(B):
        acc = core_outs[4 * b].astype(np.float32)
        for g in range(1, 4):
            acc = acc + core_outs[4 * b + g].astype(np.float32)
        out[b] = acc + bo
    return out


def kernel(query, key, value, Wq, bq, Wk, bk, Wv, bv, Wo, bo, **run_kwargs):
    nc = _get_nc()
    in_maps = make_in_maps(query, key, value, Wq, bq, Wk, bk, Wv, bv, Wo, bo)
    res = run_bass_kernel_spmd(
        nc, in_maps, core_ids=list(range(N_CORES)), **run_kwargs
    )
    out = combine_outputs([r["out"] for r in res.results], bo)
    if run_kwargs:
        kernel.last_results = res
    return out
